# revision 34
# baseline (speedup 1.0000x reference)
"""GNN message-passing kernel for 8 Trainium2 NeuronCores.

Reference computation:
    msg = x[edge_index[1]]                       # [E, 64] gather
    out = segment_sum(msg, edge_index[0], N)     # [N, 64] scatter-add

Design (v2: gather + on-chip matmul aggregation, no DMA scatter-add):
  - Destination nodes sharded across 8 cores (12500 rows/core).
  - Per core, edges are ordered by (psum-pass of dst, src-window, dst) and
    packed into 128-slot "columns".  A column's dst values span one PSUM
    bank (<=512 rows); pads use src idx 0 (finite data) and dst_rel -1.
  - SWDGE dma_gather pulls x rows (256B) for each column directly into a
    double-buffered SBUF msg buffer (no scatter DMA at all).
  - DVE builds a one-hot matrix per column: onehot[p, s] = (dst_rel[p]==s)
    via tensor_tensor(is_equal) against an iota row.  Pads (-1) match
    nothing, so garbage in pad slots is multiplied by zero columns.
  - TensorE accumulates  psum[64 feats, off:off+S] += msg[128,64]^T @ onehot
    for every column, accumulating a whole 3200-row pass of the output
    shard in PSUM (f32, start=False onto a DVE-memset region).
  - At each pass boundary DVE block-transposes PSUM (feature-major) into
    the interleaved SBUF output image (row r -> partition r%128, slot
    r//128), then the shard is written to HBM with two plain DMAs.
  - Per-core programs differ (column structure is data dependent), so 8
    separate single-core programs are built and dispatched asynchronously,
    one per device, instead of one SPMD program.
"""

import functools

import numpy as np

import concourse.bacc as bacc
import concourse.mybir as mybir

N_NODES = 100000
D = 64
N_CORES = 8
SHARD = N_NODES // N_CORES  # 12500
WIN = 32768  # int16 gather index window
N_WIN = (N_NODES + WIN - 1) // WIN  # 4
QROWS = 2048  # output rows accumulated per PSUM pass (16*128)
NQ = 7  # passes (double-buffered PSUM); last covers rows [12288, 12512)
BANK = 512  # PSUM bank capacity in f32 per partition
OH_RING = 12  # one-hot ring depth (DVE->PE decoupling)
SC_COLS = 32  # columns per super-chunk (gather/compute pipeline unit)
NBUF = 4  # msg buffer ring depth (super-chunks in flight)
CALL_COLS = 8  # max columns per dma_gather call (1024 idxs)
SCRATCH = 16384  # SWDGE desc ring: 16384/16 = 1024 descriptors
DPAD = 128  # x rows padded to 128 bf16 = 256B (gather min elem)

_f32 = mybir.dt.float32
_bf16 = mybir.dt.bfloat16
_i16 = mybir.dt.int16

OUT_SLOTS = 98  # ceil(12512/128); SBUF out image [128, 98, 64]


def _host_prep(edge_index):
    """Per-core edge packing.

    Returns list of (cfg, arrays) per core where cfg is the hashable
    program structure and arrays are the input tensors.
    """
    dst = np.asarray(edge_index[0]).astype(np.int64)
    src = np.asarray(edge_index[1]).astype(np.int64)

    per_core = []
    for c in range(N_CORES):
        m = (dst >= c * SHARD) & (dst < (c + 1) * SHARD)
        d = (dst[m] - c * SHARD).astype(np.int32)
        s = src[m].astype(np.int32)
        q = d // QROWS
        w = s // WIN
        order = np.lexsort((d, w, q))
        d, s, w, q = d[order], s[order], w[order], q[order]

        cols_meta = []  # (S, off)
        col_group = []  # (q, w) per column
        src_slots = []  # [128] int16 window-relative src per column
        rel_slots = []  # [128] f32 dst_rel per column

        # group boundaries over (q, w)
        key = q * N_WIN + w
        gb = np.flatnonzero(np.diff(key)) + 1
        starts = np.concatenate(([0], gb, [len(d)]))
        for gi in range(len(starts) - 1):
            a, b = starts[gi], starts[gi + 1]
            if a == b:
                continue
            qg, wg = int(q[a]), int(w[a])
            u = d[a:b] - qg * QROWS
            sg = (s[a:b] - wg * WIN).astype(np.int16)
            bank = u // BANK
            i = 0
            n = b - a
            while i < n:
                # cut at 128 slots or next psum bank
                j = min(i + 128, int(np.searchsorted(bank, bank[i] + 1)))
                S = int(u[j - 1] - u[i] + 1)
                off = int(u[i])
                cols_meta.append((S, off))
                col_group.append((qg, wg))
                sa = np.zeros(128, dtype=np.int16)
                ra = np.full(128, -1.0, dtype=np.float32)
                sa[: j - i] = sg[i:j]
                ra[: j - i] = (u[i:j] - u[i]).astype(np.float32)
                src_slots.append(sa)
                rel_slots.append(ra)
                i = j

        ncols = len(cols_meta)
        quarter_ncols = [0] * NQ
        for qg, _ in col_group:
            quarter_ncols[qg] += 1
        assert all(n > 0 for n in quarter_ncols), quarter_ncols

        # super-chunks: consecutive columns of one (q, w), <= SC_COLS
        sc_meta = []  # (w, ncols, (call col counts...))
        i = 0
        while i < ncols:
            g0 = col_group[i]
            j = i
            while j < ncols and j - i < SC_COLS and col_group[j] == g0:
                j += 1
            nc_cols = j - i
            calls = []
            r = nc_cols
            while r > 0:
                t = min(CALL_COLS, r)
                calls.append(t)
                r -= t
            sc_meta.append((g0[1], nc_cols, tuple(calls)))
            i = j

        cfg = (tuple(cols_meta), tuple(sc_meta), tuple(quarter_ncols))
        srcloc = np.concatenate(src_slots)  # [ncols*128] int16
        srcloc = np.ascontiguousarray(srcloc.reshape(-1, 16).T)  # [16, ncols*8]
        dstrel = np.stack(rel_slots, axis=1)  # [128, ncols] f32
        per_core.append((cfg, {"srcloc": srcloc, "dstrel": np.ascontiguousarray(dstrel)}))
    return per_core


@functools.lru_cache(maxsize=16)
def _build(cfg):
    cols_meta, sc_meta, quarter_ncols = cfg
    ncols = len(cols_meta)
    n16 = ncols * 8

    # derived structure
    col_q = []
    for qi, nq_ in enumerate(quarter_ncols):
        col_q += [qi] * nq_
    qfirst = np.concatenate(([0], np.cumsum(quarter_ncols)))  # col idx bounds
    col_sc = []  # super-chunk index per column
    col_bufj = []  # column index within its super-chunk
    calls_through = [0]  # cumulative gather calls after sc k
    cols_through = [0]  # cumulative columns after sc k
    for k, (wg, nc_cols, calls) in enumerate(sc_meta):
        col_sc += [k] * nc_cols
        col_bufj += list(range(nc_cols))
        calls_through.append(calls_through[-1] + len(calls))
        cols_through.append(cols_through[-1] + nc_cols)
    n_sc = len(sc_meta)

    nc = bacc.Bacc(None, num_swdge_queues=1, dynamic_dma_scratch_size=SCRATCH)
    x_t = nc.dram_tensor("x", [N_NODES, DPAD], _bf16, kind="ExternalInput")
    src_t = nc.dram_tensor("srcloc", [16, n16], _i16, kind="ExternalInput")
    rel_t = nc.dram_tensor("dstrel", [128, ncols], _f32, kind="ExternalInput")
    out_t = nc.dram_tensor("out", [SHARD, D], _f32, kind="ExternalOutput")

    with (
        nc.sbuf_tensor([128, n16], _i16) as src_sb,
        nc.sbuf_tensor([128, ncols], _f32) as rel_sb,
        nc.sbuf_tensor([128, BANK], _f32) as iota_sb,
        nc.sbuf_tensor([128, OH_RING, BANK], _bf16) as oh_sb,
        nc.sbuf_tensor([128, NBUF, SC_COLS, DPAD], _bf16) as msg_sb,
        nc.sbuf_tensor([128, OUT_SLOTS, D], _f32) as outb_sb,
        nc.psum_tensor("accA", [64, QROWS], _f32) as psA,
        nc.psum_tensor("accB", [64, QROWS], _f32) as psB,
        nc.semaphore("isem") as isem,
        nc.semaphore("xsem") as xsem,
        nc.semaphore("gsem0") as gsem0,
        nc.semaphore("gsem1") as gsem1,
        nc.semaphore("gsem2") as gsem2,
        nc.semaphore("gsem3") as gsem3,
        nc.semaphore("vsem") as vsem,
        nc.semaphore("mmsem") as mmsem,
        nc.semaphore("tsem") as tsem,
        nc.semaphore("ttsem") as ttsem,
        nc.semaphore("osem") as osem,
        nc.Block() as block,
    ):
        PS = (psA, psB)
        GSEMS = (gsem0, gsem1, gsem2, gsem3)
        # per-parity cumulative gather-call counts (OOO-safe thresholds)
        par_calls = [0] * NBUF
        sc_gwait = []  # (parity, threshold) per super-chunk
        for k, (wg, nc_cols, calls) in enumerate(sc_meta):
            par = k % NBUF
            par_calls[par] += len(calls)
            sc_gwait.append((par, 16 * par_calls[par]))
        PRE = 16 * 9  # 8 srcloc loads + 1 dstrel load (x16 dma sem incs)
        SLOTS_PER_PASS = QROWS // 128  # 16

        @block.gpsimd
        def _(g):
            # preload count registers for gather calls
            counts = sorted({cc * 128 for (_, _, calls) in sc_meta for cc in calls})
            regs = {}
            for v in counts:
                r = nc.alloc_register(mybir.EngineType.Pool, f"n{v}")
                g.reg_mov(r, v)
                regs[v] = r

            g.dma_start(rel_sb[:], rel_t[:]).then_inc(isem, 16)
            g.iota(
                iota_sb[:],
                [[1, BANK]],
                channel_multiplier=0,
                allow_small_or_imprecise_dtypes=True,
            ).then_inc(xsem, 1)
            for p0 in range(0, 128, 16):
                g.dma_start(src_sb[p0 : p0 + 16, :], src_t[:]).then_inc(isem, 16)
            g.wait_ge(isem, PRE)

            cg = 0  # global column cursor
            for k, (wg, nc_cols, calls) in enumerate(sc_meta):
                if k >= NBUF:
                    # buffer k%NBUF free once PE consumed super-chunk k-NBUF
                    g.wait_ge(mmsem, cols_through[k - NBUF + 1])
                hi_row = min((wg + 1) * WIN, N_NODES)
                j0 = 0
                for ccols in calls:
                    n = ccols * 128
                    g.dma_gather(
                        msg_sb[:, k % NBUF, j0 : j0 + ccols, :],
                        x_t[wg * WIN : hi_row, :],
                        src_sb[:, (cg + j0) * 8 : (cg + j0 + ccols) * 8],
                        n,
                        regs[n],
                        DPAD,
                        queue_num=0,
                    ).then_inc(GSEMS[k % NBUF], 16)
                    j0 += ccols
                cg += nc_cols

            # output writeback, one DMA per drained pass
            full = (SHARD // 128) * 128  # 12416
            nfull = full // 128  # 97
            out_v1 = out_t[0:full].rearrange("(a p) d -> p a d", p=128)
            for qi in range(NQ):
                g.wait_ge(ttsem, qi + 1)
                s0 = qi * SLOTS_PER_PASS
                s1 = min(s0 + SLOTS_PER_PASS, nfull)
                g.dma_start(
                    out_v1[:, s0:s1, :], outb_sb[:, s0:s1, :]
                ).then_inc(osem, 16)
                if qi == NQ - 1:
                    out_v2 = out_t[full:SHARD]  # [84, 64]
                    g.dma_start(
                        out_v2, outb_sb[0 : SHARD - full, nfull, :]
                    ).then_inc(osem, 16)
            g.wait_ge(osem, 16 * (NQ + 1))

        def transpose_pieces(qi):
            """The 8 strided-block transposes draining pass qi's PSUM."""
            ps = PS[qi % 2]
            qrows = QROWS if qi < NQ - 1 else 12512 - QROWS * (NQ - 1)
            nblk = qrows // 32
            pieces = []
            for mm in range(4):
                nb = len(range(mm, nblk, 4))
                if nb == 0:
                    continue
                for fh in range(2):
                    in_ap = ps.rearrange("p (a b) -> p a b", b=32)[
                        fh * 32 : fh * 32 + 32, mm:nblk:4, :
                    ]
                    s0 = SLOTS_PER_PASS * qi
                    out_ap = outb_sb[
                        32 * mm : 32 * mm + 32,
                        s0 : s0 + nb,
                        fh * 32 : fh * 32 + 32,
                    ]
                    pieces.append((out_ap, in_ap))
            return pieces

        @block.vector
        def _(v):
            v.memset(psA[:, :], 0.0).then_inc(tsem, 1)
            v.memset(psB[:, :], 0.0).then_inc(tsem, 1)
            v.wait_ge(isem, PRE)
            v.wait_ge(xsem, 1)
            pending = []  # transpose pieces of the previous pass
            pend_qi = -1
            for cg in range(ncols):
                k = col_sc[cg]
                if cg == 0 or col_sc[cg - 1] != k:
                    par, thresh = sc_gwait[k]
                    v.wait_ge(GSEMS[par], thresh)
                if cg >= OH_RING:
                    v.wait_ge(mmsem, cg - (OH_RING - 1))
                S, off = cols_meta[cg]
                v.tensor_tensor(
                    oh_sb[:, cg % OH_RING, 0:S],
                    rel_sb[:, cg : cg + 1].broadcast_to((128, S)),
                    iota_sb[:, 0:S],
                    mybir.AluOpType.is_equal,
                ).then_inc(vsem, 1)
                qi = col_q[cg]
                # interleave one pending transpose piece every few columns
                if pending and (cg - qfirst[qi]) % 16 == 15:
                    if len(pending) == 8:  # first piece: gate on pass drain
                        v.wait_ge(mmsem, int(qfirst[pend_qi + 1]))
                    out_ap, in_ap = pending.pop(0)
                    t = v.transpose(out_ap, in_ap)
                    if not pending:
                        t.then_inc(ttsem, 1)
                if cg + 1 == qfirst[qi + 1]:  # last column of pass qi
                    if pending:  # unfinished pieces of pass qi-1
                        v.wait_ge(mmsem, int(qfirst[pend_qi + 1]))
                        for j, (out_ap, in_ap) in enumerate(pending):
                            t = v.transpose(out_ap, in_ap)
                        t.then_inc(ttsem, 1)
                    pending = transpose_pieces(qi)
                    pend_qi = qi
            # drain the final pass
            v.wait_ge(mmsem, ncols)
            for out_ap, in_ap in pending:
                t = v.transpose(out_ap, in_ap)
            t.then_inc(ttsem, 1)

        @block.scalar
        def _(a):
            # ACT engine re-zeroes PSUM after each pass is drained
            for qi in range(NQ - 2):
                a.wait_ge(ttsem, qi + 1)
                a.memzero(PS[qi % 2][:, :]).then_inc(tsem, 1)

        @block.tensor
        def _(t):
            for cg in range(ncols):
                qi = col_q[cg]
                if cg == qfirst[qi]:  # first column of pass qi
                    t.wait_ge(tsem, qi + 1)
                t.wait_ge(vsem, cg + 1)
                S, off = cols_meta[cg]
                k = col_sc[cg]
                t.matmul(
                    PS[qi % 2][:, off : off + S],
                    msg_sb[:, k % NBUF, col_bufj[cg], 0:D],
                    oh_sb[:, cg % OH_RING, 0:S],
                    start=False,
                    stop=False,
                    skip_group_check=True,
                ).then_inc(mmsem, 1)

    nc.finalize()
    return nc


def _run_cores(ncs, in_maps):
    """Dispatch 8 per-core programs asynchronously, one per device."""
    import jax

    from concourse import bass2jax

    bass2jax.install_neuronx_cc_hook()
    devs = jax.devices()[: len(ncs)]
    pending = []
    for c in range(len(ncs)):
        nc = ncs[c]
        in_names, out_names, out_avals, zero_outs = _io_spec(nc)
        args = [
            jax.device_put(np.asarray(in_maps[c][name]), devs[c])
            for name in in_names
        ]
        zargs = [jax.device_put(z, devs[c]) for z in zero_outs]
        fn = _jit_for(nc, tuple(in_names), tuple(out_names), tuple(out_avals))
        pending.append((out_names, fn(*args, *zargs)))
    results = []
    for out_names, outs in pending:
        results.append({name: np.asarray(o) for name, o in zip(out_names, outs)})
    return results


def _io_spec(nc):
    partition_name = (
        nc.partition_id_tensor.name if nc.partition_id_tensor is not None else None
    )
    in_names, out_names, out_avals, zero_outs = [], [], [], []
    for alloc in nc.m.functions[0].allocations:
        if not isinstance(alloc, mybir.MemoryLocationSet):
            continue
        name = alloc.memorylocations[0].name
        if alloc.kind == "ExternalInput":
            if name != partition_name:
                in_names.append(name)
        elif alloc.kind == "ExternalOutput":
            shape = tuple(alloc.tensor_shape)
            dtype = mybir.dt.np(alloc.dtype)
            out_names.append(name)
            out_avals.append((shape, dtype))
            zero_outs.append(np.zeros(shape, dtype))
    return in_names, out_names, out_avals, zero_outs


_JIT_CACHE = {}


def _jit_for(nc, in_names, out_names, out_avals):
    key = id(nc)
    hit = _JIT_CACHE.get(key)
    if hit is not None:
        return hit
    import jax

    from concourse import bass2jax

    n_params = len(in_names)
    avals = tuple(
        jax.core.ShapedArray(shape, dtype) for shape, dtype in out_avals
    )
    all_in_names = list(in_names) + list(out_names)
    partition_name = (
        nc.partition_id_tensor.name if nc.partition_id_tensor is not None else None
    )
    if partition_name is not None:
        all_in_names.append(partition_name)

    def _body(*args):
        operands = list(args)
        if partition_name is not None:
            operands.append(bass2jax.partition_id_tensor())
        outs = bass2jax._bass_exec_p.bind(
            *operands,
            out_avals=avals,
            in_names=tuple(all_in_names),
            out_names=tuple(out_names),
            lowering_input_output_aliases=(),
            sim_require_finite=True,
            sim_require_nnan=True,
            nc=nc,
        )
        return tuple(outs)

    donate = tuple(range(n_params, n_params + len(out_names)))
    fn = jax.jit(_body, donate_argnums=donate, keep_unused=True)
    _JIT_CACHE[key] = fn
    return fn


def kernel(x, edge_index):
    x = np.ascontiguousarray(np.asarray(x), dtype=np.float32)
    bf16 = mybir.dt.np(_bf16)
    x2 = np.zeros((N_NODES, DPAD), dtype=bf16)
    x2[:, :D] = x.astype(bf16)
    per_core = _host_prep(edge_index)
    ncs = [_build(cfg) for cfg, _ in per_core]
    in_maps = [
        {"x": x2, "srcloc": arrs["srcloc"], "dstrel": arrs["dstrel"]}
        for _, arrs in per_core
    ]
    res = _run_cores(ncs, in_maps)
    out = np.concatenate([res[c]["out"] for c in range(N_CORES)])
    return out.astype(np.float32)
